# revision 13
# baseline (speedup 1.0000x reference)
"""Trainium2 Bass kernel for nn_DecoderV1 (dilated-conv decoder, 24-step recurrence).

Strategy: pure data parallel over batch (2048 -> 8 cores x 256). Inside a core,
activations live channel-major ([ch, batch] on [partitions, free]) in bf16; the
24x6 (step x layer) recurrence is emitted wavefront-ordered (w = t + l) with
blocks grouped {0},{1,2},{3,4},{5} so the cross-wavefront serial chains stay
short and pipeline across engines.

Per group: one K=128-accumulated pair of matmuls per block computes
W2.T@state + W3.T@x; tanh covers both f and g/2 halves in one [128,X]
activation; gating uses f*sigmoid(g) = 0.5W4.T(f^*g^) + 0.5W4.T f^ (0.5 folded
into W4 on host) so no sigmoid-realign op is needed. Residual/state updates are
fused scalar_tensor_tensor adds reading res straight from PSUM. Skips relu runs
on the otherwise-idle GpSimd engine into a 6-step SBUF ring consumed by
accumulating W5 matmuls; y = W6.T@relu(h)+b6 gathered into one output row.

Only the encoder tail (last d columns per dilation d, 63 of 168*6 positions) is
ever read, so the host slices/transposes it and feeds 2MB instead of 528MB.
"""
import numpy as np
import ml_dtypes

DIL = (1, 2, 4, 8, 16, 32)
T = 24
B = 2048
NC = 8
BL = B // NC          # 256 batch per core
F = 64                # filters
NW = T + len(DIL) - 1  # 29 wavefronts
GROUPS = ((0, 0), (1, 2), (3, 4), (5, 5))

_CACHE = {}


def _build():
    import concourse.bass as bass
    import concourse.tile as tile
    import concourse.mybir as mybir

    F32, BF16 = mybir.dt.float32, mybir.dt.bfloat16
    AF = mybir.ActivationFunctionType
    OP = mybir.AluOpType

    nc = bass.Bass("TRN2", target_bir_lowering=False, debug=False)

    enc_ds = [nc.dram_tensor(f"enc{l}", [F, DIL[l] * BL], BF16, kind="ExternalInput")
              for l in range(6)]
    xf_d = nc.dram_tensor("xfeat", [16, T * BL], BF16, kind="ExternalInput")
    w2_d = nc.dram_tensor("w2", [64, 128], BF16, kind="ExternalInput")
    w3_d = nc.dram_tensor("w3", [64, 128], BF16, kind="ExternalInput")
    w1_d = nc.dram_tensor("w1", [16, 64], BF16, kind="ExternalInput")
    w4_d = nc.dram_tensor("w4", [64, 128], BF16, kind="ExternalInput")
    w5_d = nc.dram_tensor("w5", [64, 6 * 128], BF16, kind="ExternalInput")
    w6_d = nc.dram_tensor("w6", [128, 1], BF16, kind="ExternalInput")
    b1_d = nc.dram_tensor("b1", [128, 1], F32, kind="ExternalInput")
    b2_d = nc.dram_tensor("b2", [128, 1], F32, kind="ExternalInput")
    b4_d = nc.dram_tensor("b4", [128, 1], F32, kind="ExternalInput")
    b4r_d = nc.dram_tensor("b4r", [64, 1], F32, kind="ExternalInput")
    b5_d = nc.dram_tensor("b5", [128, 1], F32, kind="ExternalInput")
    b6_d = nc.dram_tensor("b6", [1, 1], F32, kind="ExternalInput")
    y_d = nc.dram_tensor("y", [1, T * BL], F32, kind="ExternalOutput")

    with tile.TileContext(nc) as tc:
        with tc.tile_pool(name="const", bufs=1) as cpool, \
             tc.tile_pool(name="work", bufs=1) as wpool, \
             tc.tile_pool(name="psum", bufs=1, space="PSUM") as ppool:

            circs = [cpool.tile([F, DIL[l] * BL], BF16, name=f"circ{l}")
                     for l in range(6)]
            xfeat = cpool.tile([16, T * BL], BF16)
            w2 = cpool.tile([64, 128], BF16)
            w3 = cpool.tile([64, 128], BF16)
            w1 = cpool.tile([16, 64], BF16)
            w4 = cpool.tile([64, 128], BF16)
            w5 = cpool.tile([64, 6 * 128], BF16)
            w6 = cpool.tile([128, 1], BF16)
            b1 = cpool.tile([128, 1], F32)
            b2 = cpool.tile([128, 1], F32)
            b4 = cpool.tile([128, 1], F32)
            b4r = cpool.tile([64, 1], F32)
            b5 = cpool.tile([128, 1], F32)
            b6 = cpool.tile([1, 1], F32)
            x0_all = cpool.tile([F, T * BL], BF16)
            ring = cpool.tile([F, 6 * 6 * BL], BF16)   # slab = (w%6)*1536
            y_all = cpool.tile([1, T * BL], F32)

            for tl, dr in ([(xfeat, xf_d), (w2, w2_d), (w3, w3_d), (w1, w1_d),
                            (w4, w4_d), (w5, w5_d), (w6, w6_d), (b1, b1_d),
                            (b2, b2_d), (b4, b4_d), (b4r, b4r_d), (b5, b5_d), (b6, b6_d)]
                           + [(circs[l], enc_ds[l]) for l in range(6)]):
                nc.sync.dma_start(tl[:], dr.ap())

            from contextlib import contextmanager

            @contextmanager
            def prio(p):
                save = tc.cur_priority
                tc.cur_priority = p
                yield
                tc.cur_priority = save

            # x-history: xh[w][l*BL:(l+1)*BL] holds x_l for step t=w-l-1
            xh_tiles = {}

            def get_xh(w):
                if w not in xh_tiles:
                    xh_tiles[w] = wpool.tile([F, 5 * BL], BF16, tag="xh", bufs=17,
                                             name=f"xh{w}")
                return xh_tiles[w]

            def state_src(l, t):
                if t < DIL[l]:
                    return circs[l][:, t * BL:(t + 1) * BL]
                wsrc = (t - DIL[l]) + l + 1
                return xh_tiles[wsrc][:, l * BL:(l + 1) * BL]

            def x_src(l, t, w):
                # x_{l-1}^t
                if l == 0:
                    return x0_all[:, t * BL:(t + 1) * BL]
                return get_xh(w)[:, (l - 1) * BL:l * BL]

            def x0_chunk(c):
                with prio(-10000000 + (2 * c) * 1000 - 500):
                    xp = ppool.tile([64, 512], F32, tag="aux", bufs=2,
                                    name=f"x0p{c}")
                    nc.tensor.matmul(xp[:], w1[:], xfeat[:, c * 512:(c + 1) * 512],
                                     start=True, stop=True)
                    nc.scalar.activation(x0_all[:, c * 512:(c + 1) * 512],
                                         xp[:], AF.Tanh, bias=b1[0:64, :])

            x0_chunk(0)

            for w in range(NW):
                lmin, lmax = max(0, w - (T - 1)), min(5, w)
                ca0, ca1 = lmin * BL, (lmax + 1) * BL

                if w % 2 == 0 and w // 2 + 1 < 12:
                    x0_chunk(w // 2 + 1)

                dc = ppool.tile([128, 6 * BL], F32, tag="dc", bufs=1, name=f"dc{w}")
                outp = ppool.tile([128, 6 * BL], F32, tag="out", bufs=1, name=f"o{w}")
                th = wpool.tile([128, 6 * BL], BF16, tag="th", bufs=2, name=f"th{w}")
                ssb = wpool.tile([F, 6 * BL], BF16, tag="ssb", bufs=2, name=f"ss{w}")
                gated = wpool.tile([F, 6 * BL], BF16, tag="gated", bufs=2,
                                   name=f"gt{w}")
                rbase = (w % 6) * (6 * BL)

                for gi, (g0, g1) in enumerate(GROUPS):
                    a, b = max(g0, lmin), min(g1, lmax)
                    if a > b:
                        continue
                    c0, c1 = a * BL, (b + 1) * BL
                    # block 5 never feeds the recurrence (d=32 > T), so its
                    # gating runs on the otherwise-idle GpSimd engine
                    veng = nc.gpsimd if g0 == 5 else nc.vector
                    with prio(-1000000 + w * 1000 + gi * 12):
                        for l in range(b, a - 1, -1):
                            t = w - l
                            sl = l * BL
                            nc.tensor.matmul(dc[:, sl:sl + BL], w2[:],
                                             state_src(l, t), start=True, stop=False)
                            nc.tensor.matmul(dc[:, sl:sl + BL], w3[:],
                                             x_src(l, t, w), start=False, stop=True)
                        nc.scalar.activation(th[:, c0:c1], dc[:, c0:c1], AF.Tanh,
                                             bias=b2[:])
                        nc.vector.tensor_scalar(out=ssb[:, c0:c1],
                                                in0=th[64:128, c0:c1],
                                                scalar1=0.5, scalar2=0.5,
                                                op0=OP.mult, op1=OP.add)
                        veng.tensor_tensor(out=gated[:, c0:c1],
                                           in0=th[0:64, c0:c1],
                                           in1=ssb[:, c0:c1], op=OP.mult)
                        for l in range(b, a - 1, -1):
                            sl = l * BL
                            nc.tensor.matmul(outp[:, sl:sl + BL], w4[:],
                                             gated[:, sl:sl + BL],
                                             start=True, stop=True)
                        lf = min(b, 4)
                        if a <= lf and w + 1 < NW:
                            nxh = get_xh(w + 1)
                            # per-slice so each consumer chain waits only on
                            # its own 256-col update
                            for l in range(a, lf + 1):
                                in1 = (x0_all[:, w * BL:(w + 1) * BL] if l == 0
                                       else get_xh(w)[:, (l - 1) * BL:l * BL])
                                nc.vector.scalar_tensor_tensor(
                                    out=nxh[:, l * BL:(l + 1) * BL],
                                    in0=outp[64:128, l * BL:(l + 1) * BL],
                                    scalar=b4r[:], in1=in1,
                                    op0=OP.add, op1=OP.add)

                # skips relu -> ring slab (w%6): small DVE chunk + Act chunk so
                # neither in-order queue gets a >1us head-of-line block
                cm = min(ca0 + 512, ca1)
                nc.vector.tensor_scalar(
                    out=ring[:, rbase + ca0:rbase + cm],
                    in0=outp[0:64, ca0:cm],
                    scalar1=b4[0:64, :], scalar2=0.0, op0=OP.add, op1=OP.max)
                if cm < ca1:
                    nc.scalar.activation(ring[:, rbase + cm:rbase + ca1],
                                         outp[0:64, cm:ca1], AF.Relu,
                                         bias=b4[0:64, :])

                # step completion
                s = w - 5
                if s >= 0:
                    hp = ppool.tile([128, BL], F32, tag="aux", bufs=2, name=f"hp{s}")
                    for l in range(6):
                        base = ((s + l) % 6) * (6 * BL)
                        nc.tensor.matmul(hp[:], w5[:, l * 128:(l + 1) * 128],
                                         ring[:, base + l * BL:base + (l + 1) * BL],
                                         start=(l == 0), stop=(l == 5))
                    hsb = wpool.tile([128, BL], BF16, tag="hsb", bufs=3, name=f"hs{s}")
                    nc.scalar.activation(hsb[:], hp[:], AF.Relu, bias=b5[:])
                    yp = ppool.tile([1, BL], F32, tag="aux", bufs=2, name=f"yp{s}")
                    nc.tensor.matmul(yp[:], w6[:], hsb[:], start=True, stop=True)
                    nc.scalar.activation(y_all[:, s * BL:(s + 1) * BL], yp[:],
                                         AF.Identity, bias=b6[:])

            nc.sync.dma_start(y_d.ap(), y_all[:])

    _split_multi_waits(nc)
    return nc


def _split_multi_waits(nc, max_waits: int = 1) -> int:
    """This walrus build encodes at most one sync wait per instruction; hoist
    extras onto same-engine EventSemaphore wait-nops (as raw bass emits)."""
    import concourse.mybir as mybir
    n = 0
    for f in nc.m.functions:
        for bb in f.blocks:
            insts = bb.instructions
            if not any(i.sync_info and i.sync_info.on_wait
                       and len(i.sync_info.on_wait) > max_waits for i in insts):
                continue
            new = []
            for inst in insts:
                si = inst.sync_info
                if si is not None and si.on_wait and len(si.on_wait) > max_waits:
                    waits = list(si.on_wait)
                    for j, wt in enumerate(waits[:-max_waits]):
                        new.append(mybir.InstEventSemaphore(
                            name=f"{inst.name}_xw{j}", engine=inst.engine,
                            sync_info=mybir.SyncInfo(on_wait=[wt], on_update=[])))
                        n += 1
                    inst.sync_info = mybir.SyncInfo(
                        on_wait=waits[-max_waits:], on_update=list(si.on_update))
                new.append(inst)
            bb.instructions = new
    return n


def _prep_inputs(inputs):
    bf = ml_dtypes.bfloat16
    enc = np.asarray(inputs["encoder_outputs"], np.float32)   # [6,2048,168,64]
    df = np.asarray(inputs["decoder_features"], np.float32)   # [2048,24,15]
    di = np.asarray(inputs["decoder_init_input"], np.float32)  # [2048,1]

    W = {k: np.asarray(inputs[k], np.float32) for k in
         ["W1", "W2", "W3", "W4", "W5", "W6", "b1", "b2", "b4", "b5", "b6"]}
    w2 = W["W2"].copy(); w2[:, 64:128] *= 0.5   # sigmoid(g)=0.5*tanh(g/2)+0.5
    w3 = W["W3"].copy(); w3[:, 64:128] *= 0.5
    w4 = W["W4"]
    w5 = np.concatenate([W["W5"][l * 64:(l + 1) * 64, :] for l in range(6)],
                        axis=1).astype(bf)                            # [64, 768]
    b1 = np.zeros((128, 1), np.float32); b1[0:64, 0] = W["b1"]
    b2 = W["b2"].reshape(128, 1).astype(np.float32).copy(); b2[64:128] *= 0.5
    b4 = W["b4"].reshape(128, 1).astype(np.float32)
    b5 = W["b5"].reshape(128, 1).astype(np.float32)
    b6 = W["b6"].reshape(1, 1).astype(np.float32)

    in_maps = []
    for c in range(NC):
        bs = slice(c * BL, (c + 1) * BL)
        m = {
            "xfeat": None,
            "w2": w2.astype(bf), "w3": w3.astype(bf), "w1": W["W1"].astype(bf),
            "w4": w4.astype(bf), "w5": w5, "w6": W["W6"].astype(bf),
            "b1": b1, "b2": b2, "b4": b4, "b4r": np.ascontiguousarray(b4[64:128]),
            "b5": b5, "b6": b6,
        }
        for l, d in enumerate(DIL):
            blk = np.ascontiguousarray(
                np.transpose(enc[l, bs, 168 - d:168, :], (2, 1, 0)))  # [F, d, BL]
            m[f"enc{l}"] = blk.reshape(F, d * BL).astype(bf)
        xf = np.empty((16, T * BL), np.float32)
        xf[0] = np.repeat(di[bs, 0][None, :], T, axis=0).reshape(T * BL)
        xf[1:16] = df[bs].transpose(2, 1, 0).reshape(15, T * BL)
        m["xfeat"] = xf.astype(bf)
        in_maps.append(m)
    return in_maps


def kernel(**inputs) -> np.ndarray:
    from concourse.bass_utils import run_bass_kernel_spmd
    if "nc" not in _CACHE:
        _CACHE["nc"] = _build()
    nc = _CACHE["nc"]
    in_maps = _prep_inputs(inputs)
    res = run_bass_kernel_spmd(nc, in_maps, core_ids=list(range(NC)))
    out = np.empty((B, T, 1), np.float32)
    for c in range(NC):
        y = res.results[c]["y"].reshape(T, BL)  # [t, b]
        out[c * BL:(c + 1) * BL, :, 0] = y.T
    return out


# revision 14
# speedup vs baseline: 1.0876x; 1.0876x over previous
"""Trainium2 Bass kernel for nn_DecoderV1 (dilated-conv decoder, 24-step recurrence).

Strategy: pure data parallel over batch (2048 -> 8 cores x 256). Inside a core,
activations live channel-major ([ch, batch] on [partitions, free]) in bf16; the
24x6 (step x layer) recurrence is emitted wavefront-ordered (w = t + l) with
blocks grouped {0},{1,2},{3,4},{5} so the cross-wavefront serial chains stay
short and pipeline across engines.

Per group: one K=128-accumulated pair of matmuls per block computes
W2.T@state + W3.T@x; tanh covers both f and g/2 halves in one [128,X]
activation; gating uses f*sigmoid(g) = 0.5W4.T(f^*g^) + 0.5W4.T f^ (0.5 folded
into W4 on host) so no sigmoid-realign op is needed. Residual/state updates are
fused scalar_tensor_tensor adds reading res straight from PSUM. Skips relu runs
on the otherwise-idle GpSimd engine into a 6-step SBUF ring consumed by
accumulating W5 matmuls; y = W6.T@relu(h)+b6 gathered into one output row.

Only the encoder tail (last d columns per dilation d, 63 of 168*6 positions) is
ever read, so the host slices/transposes it and feeds 2MB instead of 528MB.
"""
import numpy as np
import ml_dtypes

DIL = (1, 2, 4, 8, 16, 32)
T = 24
B = 2048
NC = 8
BL = B // NC          # 256 batch per core
F = 64                # filters
NW = T + len(DIL) - 1  # 29 wavefronts
GROUPS = ((0, 0), (1, 2), (3, 4), (5, 5))

_CACHE = {}


def _build():
    import concourse.bass as bass
    import concourse.tile as tile
    import concourse.mybir as mybir

    F32, BF16 = mybir.dt.float32, mybir.dt.bfloat16
    AF = mybir.ActivationFunctionType
    OP = mybir.AluOpType

    nc = bass.Bass("TRN2", target_bir_lowering=False, debug=False)

    enc_ds = [nc.dram_tensor(f"enc{l}", [F, DIL[l] * BL], BF16, kind="ExternalInput")
              for l in range(6)]
    xf_d = nc.dram_tensor("xfeat", [16, T * BL], BF16, kind="ExternalInput")
    w2_d = nc.dram_tensor("w2", [64, 128], BF16, kind="ExternalInput")
    w3_d = nc.dram_tensor("w3", [64, 128], BF16, kind="ExternalInput")
    w1_d = nc.dram_tensor("w1", [16, 64], BF16, kind="ExternalInput")
    w4_d = nc.dram_tensor("w4", [64, 128], BF16, kind="ExternalInput")
    w5_d = nc.dram_tensor("w5", [64, 6 * 128], BF16, kind="ExternalInput")
    w6_d = nc.dram_tensor("w6", [128, 1], BF16, kind="ExternalInput")
    b1_d = nc.dram_tensor("b1", [128, 1], F32, kind="ExternalInput")
    b2_d = nc.dram_tensor("b2", [128, 1], F32, kind="ExternalInput")
    b4_d = nc.dram_tensor("b4", [128, 1], F32, kind="ExternalInput")
    b4r_d = nc.dram_tensor("b4r", [64, 1], F32, kind="ExternalInput")
    b5_d = nc.dram_tensor("b5", [128, 1], F32, kind="ExternalInput")
    b6_d = nc.dram_tensor("b6", [1, 1], F32, kind="ExternalInput")
    y_d = nc.dram_tensor("y", [1, T * BL], F32, kind="ExternalOutput")

    with tile.TileContext(nc) as tc:
        with tc.tile_pool(name="const", bufs=1) as cpool, \
             tc.tile_pool(name="work", bufs=1) as wpool, \
             tc.tile_pool(name="psum", bufs=1, space="PSUM") as ppool:

            circs = [cpool.tile([F, DIL[l] * BL], BF16, name=f"circ{l}")
                     for l in range(6)]
            xfeat = cpool.tile([16, T * BL], BF16)
            w2 = cpool.tile([64, 128], BF16)
            w3 = cpool.tile([64, 128], BF16)
            w1 = cpool.tile([16, 64], BF16)
            w4 = cpool.tile([64, 128], BF16)
            w5 = cpool.tile([64, 6 * 128], BF16)
            w6 = cpool.tile([128, 1], BF16)
            b1 = cpool.tile([128, 1], F32)
            b2 = cpool.tile([128, 1], F32)
            b4 = cpool.tile([128, 1], F32)
            b4r = cpool.tile([64, 1], F32)
            b5 = cpool.tile([128, 1], F32)
            b6 = cpool.tile([1, 1], F32)
            x0_all = cpool.tile([F, T * BL], BF16)
            ring = cpool.tile([F, 6 * 6 * BL], BF16)   # slab = (w%6)*1536
            y_all = cpool.tile([1, T * BL], F32)

            for tl, dr in ([(xfeat, xf_d), (w2, w2_d), (w3, w3_d), (w1, w1_d),
                            (w4, w4_d), (w5, w5_d), (w6, w6_d), (b1, b1_d),
                            (b2, b2_d), (b4, b4_d), (b4r, b4r_d), (b5, b5_d), (b6, b6_d)]
                           + [(circs[l], enc_ds[l]) for l in range(6)]):
                nc.sync.dma_start(tl[:], dr.ap())

            from contextlib import contextmanager

            @contextmanager
            def prio(p):
                save = tc.cur_priority
                tc.cur_priority = p
                yield
                tc.cur_priority = save

            # x-history: xh[w][l*BL:(l+1)*BL] holds x_l for step t=w-l-1
            xh_tiles = {}

            def get_xh(w):
                if w not in xh_tiles:
                    xh_tiles[w] = wpool.tile([F, 5 * BL], BF16, tag="xh", bufs=17,
                                             name=f"xh{w}")
                return xh_tiles[w]

            def state_src(l, t):
                if t < DIL[l]:
                    return circs[l][:, t * BL:(t + 1) * BL]
                wsrc = (t - DIL[l]) + l + 1
                return xh_tiles[wsrc][:, l * BL:(l + 1) * BL]

            def x_src(l, t, w):
                # x_{l-1}^t
                if l == 0:
                    return x0_all[:, t * BL:(t + 1) * BL]
                return get_xh(w)[:, (l - 1) * BL:l * BL]

            def x0_chunk(c):
                with prio(-10000000 + (2 * c) * 1000 - 500):
                    xp = ppool.tile([64, 512], F32, tag="aux", bufs=2,
                                    name=f"x0p{c}")
                    nc.tensor.matmul(xp[:], w1[:], xfeat[:, c * 512:(c + 1) * 512],
                                     start=True, stop=True)
                    nc.scalar.activation(x0_all[:, c * 512:(c + 1) * 512],
                                         xp[:], AF.Tanh, bias=b1[0:64, :])

            x0_chunk(0)

            for w in range(NW):
                lmin, lmax = max(0, w - (T - 1)), min(5, w)
                ca0, ca1 = lmin * BL, (lmax + 1) * BL

                if w % 2 == 0 and w // 2 + 1 < 12:
                    x0_chunk(w // 2 + 1)

                dc = ppool.tile([128, 6 * BL], F32, tag="dc", bufs=1, name=f"dc{w}")
                outp = ppool.tile([128, 6 * BL], F32, tag="out", bufs=1, name=f"o{w}")
                th = wpool.tile([128, 6 * BL], BF16, tag="th", bufs=2, name=f"th{w}")
                ssb = wpool.tile([F, 6 * BL], BF16, tag="ssb", bufs=2, name=f"ss{w}")
                gated = wpool.tile([F, 6 * BL], BF16, tag="gated", bufs=2,
                                   name=f"gt{w}")
                rbase = (w % 6) * (6 * BL)

                for gi, (g0, g1) in enumerate(GROUPS):
                    a, b = max(g0, lmin), min(g1, lmax)
                    if a > b:
                        continue
                    c0, c1 = a * BL, (b + 1) * BL
                    # block 5 never feeds the recurrence (d=32 > T), so its
                    # gating runs on the otherwise-idle GpSimd engine
                    veng = nc.gpsimd if g0 == 5 else nc.vector
                    with prio(-1000000 + w * 1000 + gi * 12):
                        for l in range(b, a - 1, -1):
                            t = w - l
                            sl = l * BL
                            nc.tensor.matmul(dc[:, sl:sl + BL], w2[:],
                                             state_src(l, t), start=True, stop=False)
                            nc.tensor.matmul(dc[:, sl:sl + BL], w3[:],
                                             x_src(l, t, w), start=False, stop=True)
                        nc.scalar.activation(th[:, c0:c1], dc[:, c0:c1], AF.Tanh,
                                             bias=b2[:])
                        nc.vector.tensor_scalar(out=ssb[:, c0:c1],
                                                in0=th[64:128, c0:c1],
                                                scalar1=0.5, scalar2=0.5,
                                                op0=OP.mult, op1=OP.add)
                        veng.tensor_tensor(out=gated[:, c0:c1],
                                           in0=th[0:64, c0:c1],
                                           in1=ssb[:, c0:c1], op=OP.mult)
                        for l in range(b, a - 1, -1):
                            sl = l * BL
                            nc.tensor.matmul(outp[:, sl:sl + BL], w4[:],
                                             gated[:, sl:sl + BL],
                                             start=True, stop=True)
                        lf = min(b, 4)
                        if a <= lf and w + 1 < NW:
                            nxh = get_xh(w + 1)
                            in1 = (x0_all[:, w * BL:(w + 1) * BL] if a == 0
                                   else get_xh(w)[:, (a - 1) * BL:lf * BL])
                            nc.vector.scalar_tensor_tensor(
                                out=nxh[:, a * BL:(lf + 1) * BL],
                                in0=outp[64:128, a * BL:(lf + 1) * BL],
                                scalar=b4r[:], in1=in1,
                                op0=OP.add, op1=OP.add)

                # skips relu -> ring slab (w%6): small DVE chunk + Act chunk so
                # neither in-order queue gets a >1us head-of-line block
                cm = min(ca0 + 512, ca1)
                nc.vector.tensor_scalar(
                    out=ring[:, rbase + ca0:rbase + cm],
                    in0=outp[0:64, ca0:cm],
                    scalar1=b4[0:64, :], scalar2=0.0, op0=OP.add, op1=OP.max)
                if cm < ca1:
                    nc.scalar.activation(ring[:, rbase + cm:rbase + ca1],
                                         outp[0:64, cm:ca1], AF.Relu,
                                         bias=b4[0:64, :])

                # step completion
                s = w - 5
                if s >= 0:
                    hp = ppool.tile([128, BL], F32, tag="aux", bufs=2, name=f"hp{s}")
                    for l in range(6):
                        base = ((s + l) % 6) * (6 * BL)
                        nc.tensor.matmul(hp[:], w5[:, l * 128:(l + 1) * 128],
                                         ring[:, base + l * BL:base + (l + 1) * BL],
                                         start=(l == 0), stop=(l == 5))
                    hsb = wpool.tile([128, BL], BF16, tag="hsb", bufs=3, name=f"hs{s}")
                    nc.scalar.activation(hsb[:], hp[:], AF.Relu, bias=b5[:])
                    yp = ppool.tile([1, BL], F32, tag="aux", bufs=2, name=f"yp{s}")
                    nc.tensor.matmul(yp[:], w6[:], hsb[:], start=True, stop=True)
                    nc.scalar.activation(y_all[:, s * BL:(s + 1) * BL], yp[:],
                                         AF.Identity, bias=b6[:])

            nc.sync.dma_start(y_d.ap(), y_all[:])

    _split_multi_waits(nc)
    return nc


def _split_multi_waits(nc, max_waits: int = 1) -> int:
    """This walrus build encodes at most one sync wait per instruction; hoist
    extras onto same-engine EventSemaphore wait-nops (as raw bass emits)."""
    import concourse.mybir as mybir
    n = 0
    for f in nc.m.functions:
        for bb in f.blocks:
            insts = bb.instructions
            if not any(i.sync_info and i.sync_info.on_wait
                       and len(i.sync_info.on_wait) > max_waits for i in insts):
                continue
            new = []
            for inst in insts:
                si = inst.sync_info
                if si is not None and si.on_wait and len(si.on_wait) > max_waits:
                    waits = list(si.on_wait)
                    for j, wt in enumerate(waits[:-max_waits]):
                        new.append(mybir.InstEventSemaphore(
                            name=f"{inst.name}_xw{j}", engine=inst.engine,
                            sync_info=mybir.SyncInfo(on_wait=[wt], on_update=[])))
                        n += 1
                    inst.sync_info = mybir.SyncInfo(
                        on_wait=waits[-max_waits:], on_update=list(si.on_update))
                new.append(inst)
            bb.instructions = new
    return n


def _prep_inputs(inputs):
    bf = ml_dtypes.bfloat16
    enc = np.asarray(inputs["encoder_outputs"], np.float32)   # [6,2048,168,64]
    df = np.asarray(inputs["decoder_features"], np.float32)   # [2048,24,15]
    di = np.asarray(inputs["decoder_init_input"], np.float32)  # [2048,1]

    W = {k: np.asarray(inputs[k], np.float32) for k in
         ["W1", "W2", "W3", "W4", "W5", "W6", "b1", "b2", "b4", "b5", "b6"]}
    w2 = W["W2"].copy(); w2[:, 64:128] *= 0.5   # sigmoid(g)=0.5*tanh(g/2)+0.5
    w3 = W["W3"].copy(); w3[:, 64:128] *= 0.5
    w4 = W["W4"]
    w5 = np.concatenate([W["W5"][l * 64:(l + 1) * 64, :] for l in range(6)],
                        axis=1).astype(bf)                            # [64, 768]
    b1 = np.zeros((128, 1), np.float32); b1[0:64, 0] = W["b1"]
    b2 = W["b2"].reshape(128, 1).astype(np.float32).copy(); b2[64:128] *= 0.5
    b4 = W["b4"].reshape(128, 1).astype(np.float32)
    b5 = W["b5"].reshape(128, 1).astype(np.float32)
    b6 = W["b6"].reshape(1, 1).astype(np.float32)

    in_maps = []
    for c in range(NC):
        bs = slice(c * BL, (c + 1) * BL)
        m = {
            "xfeat": None,
            "w2": w2.astype(bf), "w3": w3.astype(bf), "w1": W["W1"].astype(bf),
            "w4": w4.astype(bf), "w5": w5, "w6": W["W6"].astype(bf),
            "b1": b1, "b2": b2, "b4": b4, "b4r": np.ascontiguousarray(b4[64:128]),
            "b5": b5, "b6": b6,
        }
        for l, d in enumerate(DIL):
            blk = np.ascontiguousarray(
                np.transpose(enc[l, bs, 168 - d:168, :], (2, 1, 0)))  # [F, d, BL]
            m[f"enc{l}"] = blk.reshape(F, d * BL).astype(bf)
        xf = np.empty((16, T * BL), np.float32)
        xf[0] = np.repeat(di[bs, 0][None, :], T, axis=0).reshape(T * BL)
        xf[1:16] = df[bs].transpose(2, 1, 0).reshape(15, T * BL)
        m["xfeat"] = xf.astype(bf)
        in_maps.append(m)
    return in_maps


def kernel(**inputs) -> np.ndarray:
    from concourse.bass_utils import run_bass_kernel_spmd
    if "nc" not in _CACHE:
        _CACHE["nc"] = _build()
    nc = _CACHE["nc"]
    in_maps = _prep_inputs(inputs)
    res = run_bass_kernel_spmd(nc, in_maps, core_ids=list(range(NC)))
    out = np.empty((B, T, 1), np.float32)
    for c in range(NC):
        y = res.results[c]["y"].reshape(T, BL)  # [t, b]
        out[c * BL:(c + 1) * BL, :, 0] = y.T
    return out
